# revision 45
# baseline (speedup 1.0000x reference)
"""Trainium2 Bass kernel for nn_GraphVToS_9388798509586 (gnn_message_passing).

Contract: kernel(**inputs) takes FULL unsharded numpy inputs and returns the
FULL [8, 128, 128, 64] float32 output.

Math (per batch element b, with F=64, K=64, C=3, N=128):
    pi = vf @ w_vs[:F]            # [N, C, K]
    pj = vf @ w_vs[F:] + b_vs     # [N, C, K]  (bias folds exactly: b*sum_c d)
    s[i,j,k] = sum_c d[i,j,c] * (pi[i,c,k] + pj[j,c,k])
    out      = relu(s)

Sharding: data-parallel over batch B=8, one batch element per NeuronCore.

Device kernel design (per core). Both terms run on the TensorEngine as
grouped block-diagonal matmuls over groups of G=8 rows (contract dim
(row,c)=24), with the tiny O(N*C*K) projections and the block-diagonal
weight expansion prepacked on the host (pure input relayout; all N^2-scale
FLOPs stay on device):

  * term2[i,(j,k)] = sum_c d[i,j,c]*pj[j,c,k]: per j-group matmul,
    lhsT = d^T chunk [24, 128i], moving = block-diag pj [24, 8*64=512].
    Output lands directly in the final [i, (j,k)] layout.
  * term1[j,(i,k)] = sum_c d[i,j,c]*pi[i,c,k] is only a matmul in the
    TRANSPOSED layout (j on partitions). It is computed there, then the
    (i<->j) layout fix goes through a DRAM scratch round-trip: contiguous
    per-partition scatter [j,(i,k)] -> scratch[j,i,k], then strided
    readback [i,(j,k)] (128B chunks). DMA through DRAM is the only
    cross-partition reorder that doesn't burn compute-engine time.
  * combine: DVE adds term2-psum + term1-readback into bf16, ACT relu,
    contiguous output DMA. Output is bf16, upcast to f32 on the host
    (rel-err budget 2e-2; measured ~5e-3).

Input packing: one [128, 5120] bf16 tensor; four 24-row stripes at
partition bases 0/32/64/96 (legal quadrant bases for 24-row matmul
operands) hold {term2 groups 0-7, term2 groups 8-15, term1 groups 0-7,
term1 groups 8-15}; within a stripe each group is [ltw_g(128) | wdg_g(512)]
so streaming column-chunk DMAs unlock groups progressively.
"""

import numpy as np

B, N, C, F, K = 8, 128, 3, 64, 64
_N_CORES = 8
_G = 8                   # group size (rows per block-diag group)
_NG = N // _G            # 16 groups
_CTR = _G * C            # 24 contraction rows per group
_GCOL = N + _G * K       # 640 cols per group in the packed input
_W = (_NG // 2) * _GCOL  # 5120 cols per stripe

_cached = {}


def _build_nc():
    import concourse.mybir as mybir
    import concourse.tile as tile
    from concourse import bacc

    fp32 = mybir.dt.float32
    bf16 = mybir.dt.bfloat16

    GW = _G * K  # 512

    nc = bacc.Bacc(None)
    din_d = nc.dram_tensor("din", [N, _W], bf16, kind="ExternalInput")
    out_d = nc.dram_tensor("out", [N, N * K], bf16, kind="ExternalOutput")
    scr_d = nc.dram_tensor("scr", [N, N * K], bf16, kind="Internal")

    def op_slices(side, g):
        """(lhsT, rhs, tile_position) for side (0=term2, 1=term1), group g."""
        base = 32 * (2 * side + (g >= 8))
        col = (g % 8) * _GCOL
        return (
            din_t[base : base + _CTR, col : col + N],
            din_t[base : base + _CTR, col + N : col + _GCOL],
            (base, 0),
        )

    with tile.TileContext(nc) as tc:
        with (
            tc.tile_pool(name="persist", bufs=1) as pp,
            tc.tile_pool(name="acc", bufs=8) as ap,
            tc.tile_pool(name="rb", bufs=4) as rp,
            tc.tile_pool(name="psumA", bufs=4, space="PSUM") as qpa,
            tc.tile_pool(name="psumC", bufs=4, space="PSUM") as qpc,
        ):
            din_t = pp.tile([N, _W], bf16, tag="din")
            t1 = pp.tile([N, N * K], bf16, tag="t1")
            scr_flat = scr_d.rearrange("a b -> (a b)")

            # Streaming input: 8 column chunks (1 group per stripe each).
            for h in range(8):
                cw = _W // 8
                nc.sync.dma_start(
                    din_t[:, h * cw : (h + 1) * cw], din_d[:, h * cw : (h + 1) * cw]
                )

            # HAM warm-up: dense dummy matmuls during the input-DMA window
            # flip the PE clock gate to 2.4 GHz before phase A begins
            # (~600ns -> ~380ns per N=512 matmul).
            wi = pp.tile([32, 640], bf16, tag="warm")
            nc.gpsimd.memset(wi[:], 0.0)
            wp = qpa.tile([N, 512], fp32, tag="psA")
            for _ in range(10):
                nc.tensor.matmul(
                    wp[:], wi[:, 0:128], wi[:, 128:640],
                    start=True, stop=True, tile_position=(0, 0),
                )

            # Phase A: term1 in transposed layout [j, (i,k)] -> t1 (bf16),
            # casts alternating DVE/ACT; scatter to scratch[j, i, k] after
            # every 2 groups.
            for g in range(_NG):
                lhsT, rhs, tpos = op_slices(1, g)
                ps = qpa.tile([N, GW], fp32, tag="psA")
                nc.tensor.matmul(
                    ps[:], lhsT, rhs, start=True, stop=True, tile_position=tpos
                )
                sl = t1[:, g * GW : (g + 1) * GW]
                if g % 2 == 0:
                    nc.vector.tensor_copy(sl, ps[:])
                else:
                    nc.scalar.copy(sl, ps[:])
                if g % 2 == 1:
                    nc.sync.dma_start(
                        scr_d[:, (g - 1) * GW : (g + 1) * GW],
                        t1[:, (g - 1) * GW : (g + 1) * GW],
                    )

            # Keep the PE clock warm across the scatter/readback barrier.
            wp2 = qpa.tile([N, 512], fp32, tag="psA")
            for _ in range(6):
                nc.tensor.matmul(
                    wp2[:], wi[:, 0:128], wi[:, 128:640],
                    start=True, stop=True, tile_position=(0, 0),
                )

            # Phase C: per pair of groups: strided readback [i, (j16,k)]
            # (128B chunks); per group: term2 matmul, DVE add, relu; store.
            for gp in range(_NG // 2):
                rb = rp.tile([N, 2 * GW], bf16, tag="rb")
                rbv = rb.rearrange("a (j k) -> a j k", j=2 * _G)
                src = scr_flat.rearrange("(j i k) -> i j k", j=N, i=N, k=K)[
                    :, gp * 2 * _G : (gp + 1) * 2 * _G, :
                ]
                # Alternate the two HWDGE rings so descriptor generation for
                # consecutive readbacks proceeds in parallel.
                (nc.sync if gp % 2 == 0 else nc.scalar).dma_start(rbv[:], src)

                acc = ap.tile([N, 2 * GW], bf16, tag="acc")
                for q in range(2):
                    g = 2 * gp + q
                    lhsT, rhs, tpos = op_slices(0, g)
                    ps = qpc.tile([N, GW], fp32, tag="psC")
                    nc.tensor.matmul(
                        ps[:], lhsT, rhs, start=True, stop=True, tile_position=tpos
                    )
                    nc.vector.tensor_tensor(
                        acc[:, q * GW : (q + 1) * GW],
                        ps[:],
                        rb[:, q * GW : (q + 1) * GW],
                        mybir.AluOpType.add,
                    )
                # relu on DVE (427ns vs 1147ns on ACT at this width); output
                # DMA issued on the ACT HWDGE ring to keep SP free for rbs.
                sl = acc[:]
                nc.vector.tensor_scalar_max(sl, sl, 0.0)
                nc.scalar.dma_start(
                    out_d[:, gp * 2 * GW : (gp + 1) * 2 * GW], acc[:]
                )
    nc.finalize()
    return nc


def _host_pack(vf, d, w, b):
    """Per-batch host prepack -> list of per-core input dicts (bf16)."""
    import ml_dtypes

    bf = ml_dtypes.bfloat16
    w_i, w_j = w[:F], w[F:]
    pi = np.einsum("bncf,fk->bnck", vf, w_i, optimize=True)
    pj = np.einsum("bncf,fk->bnck", vf, w_j, optimize=True) + b

    def pack_side(dT, proj):
        """dT [(row,c), col] grouped by G rows; proj [row, c, k].

        Returns [2, 24, 5120]: two 8-group stripes, each group packed as
        [ltw_g (128 cols) | wdg_g (512 cols)].
        """
        out = np.zeros((2, _CTR, _W), np.float32)
        rr = np.arange(_G)
        for g in range(_NG):
            s, gg = divmod(g, 8)
            col = gg * _GCOL
            # ltw_g[(r*3+c), col] = d[row=g*G+r, col, c]
            out[s, :, col : col + N] = dT[g * _CTR : (g + 1) * _CTR]
            # wdg_g[(r*3+c), r*64+k] = proj[g*G+r, c, k]
            blk = np.zeros((_CTR, _G, K), np.float32)
            for c in range(C):
                blk[rr * C + c, rr, :] = proj[g * _G + rr, c, :]
            out[s, :, col + N : col + _GCOL] = blk.reshape(_CTR, _G * K)
        return out

    in_maps = []
    for bi in range(B):
        A2 = d[bi].transpose(1, 2, 0).reshape(N * C, N)  # [(j,c), i]
        A1 = d[bi].transpose(0, 2, 1).reshape(N * C, N)  # [(i,c), j]
        s2 = pack_side(A2, pj[bi])
        s1 = pack_side(A1, pi[bi])
        din = np.zeros((N, _W), np.float32)
        din[0:_CTR] = s2[0]
        din[32 : 32 + _CTR] = s2[1]
        din[64 : 64 + _CTR] = s1[0]
        din[96 : 96 + _CTR] = s1[1]
        in_maps.append({"din": np.ascontiguousarray(din).astype(bf)})
    return in_maps


def _run(in_maps, trace=False, **kw):
    from concourse.bass_utils import run_bass_kernel_spmd

    if "nc" not in _cached:
        _cached["nc"] = _build_nc()
    return run_bass_kernel_spmd(
        _cached["nc"], in_maps, core_ids=list(range(_N_CORES)), trace=trace, **kw
    )


def kernel(**inputs: np.ndarray) -> np.ndarray:
    vf = np.asarray(inputs["vector_features"], np.float32)
    d = np.asarray(inputs["distances"], np.float32)
    w = np.asarray(inputs["w_vs"], np.float32)
    b = np.asarray(inputs["b_vs"], np.float32)

    in_maps = _host_pack(vf, d, w, b)
    res = _run(in_maps)
    out = np.stack([r["out"] for r in res.results])  # [B, N, N*K] bf16
    return out.reshape(B, N, N, K).astype(np.float32)


if __name__ == "__main__":
    rng = np.random.default_rng(0)
    ins = {
        "vector_features": rng.standard_normal((B, N, C, F)).astype(np.float32),
        "distances": rng.standard_normal((B, N, N, C)).astype(np.float32),
        "w_vs": (rng.standard_normal((2 * F, K)) / np.sqrt(2 * F)).astype(np.float32),
        "b_vs": np.zeros(K, np.float32),
    }
    out = kernel(**ins)
    pi = np.einsum("bncf,fk->bnck", ins["vector_features"], ins["w_vs"][:F])
    pj = np.einsum("bncf,fk->bnck", ins["vector_features"], ins["w_vs"][F:])
    s = np.einsum("bick,bijc->bijk", pi, ins["distances"]) + np.einsum(
        "bjck,bijc->bijk", pj, ins["distances"]
    )
    want = np.maximum(s, 0)
    rel = np.abs(out - want).max() / np.abs(want).max()
    print("rel err vs numpy:", rel)


# revision 46
# speedup vs baseline: 1.0423x; 1.0423x over previous
"""Trainium2 Bass kernel for nn_GraphVToS_9388798509586 (gnn_message_passing).

Contract: kernel(**inputs) takes FULL unsharded numpy inputs and returns the
FULL [8, 128, 128, 64] float32 output.

Math (per batch element b, with F=64, K=64, C=3, N=128):
    pi = vf @ w_vs[:F]            # [N, C, K]
    pj = vf @ w_vs[F:] + b_vs     # [N, C, K]  (bias folds exactly: b*sum_c d)
    s[i,j,k] = sum_c d[i,j,c] * (pi[i,c,k] + pj[j,c,k])
    out      = relu(s)

Sharding: data-parallel over batch B=8, one batch element per NeuronCore.

Device kernel design (per core). Both terms run on the TensorEngine as
grouped block-diagonal matmuls over groups of G=8 rows (contract dim
(row,c)=24), with the tiny O(N*C*K) projections and the block-diagonal
weight expansion prepacked on the host (pure input relayout; all N^2-scale
FLOPs stay on device):

  * term2[i,(j,k)] = sum_c d[i,j,c]*pj[j,c,k]: per j-group matmul,
    lhsT = d^T chunk [24, 128i], moving = block-diag pj [24, 8*64=512].
    Output lands directly in the final [i, (j,k)] layout.
  * term1[j,(i,k)] = sum_c d[i,j,c]*pi[i,c,k] is only a matmul in the
    TRANSPOSED layout (j on partitions). It is computed there, then the
    (i<->j) layout fix goes through a DRAM scratch round-trip: contiguous
    per-partition scatter [j,(i,k)] -> scratch[j,i,k], then strided
    readback [i,(j,k)] (128B chunks). DMA through DRAM is the only
    cross-partition reorder that doesn't burn compute-engine time.
  * combine: DVE adds term2-psum + term1-readback into bf16, ACT relu,
    contiguous output DMA. Output is bf16, upcast to f32 on the host
    (rel-err budget 2e-2; measured ~5e-3).

Input packing: one [128, 5120] bf16 tensor; four 24-row stripes at
partition bases 0/32/64/96 (legal quadrant bases for 24-row matmul
operands) hold {term2 groups 0-7, term2 groups 8-15, term1 groups 0-7,
term1 groups 8-15}; within a stripe each group is [ltw_g(128) | wdg_g(512)]
so streaming column-chunk DMAs unlock groups progressively.
"""

import numpy as np

B, N, C, F, K = 8, 128, 3, 64, 64
_N_CORES = 8
_G = 8                   # group size (rows per block-diag group)
_NG = N // _G            # 16 groups
_CTR = _G * C            # 24 contraction rows per group
_GCOL = N + _G * K       # 640 cols per group in the packed input
_W = (_NG // 2) * _GCOL  # 5120 cols per stripe

_cached = {}


def _build_nc():
    import concourse.mybir as mybir
    import concourse.tile as tile
    from concourse import bacc

    fp32 = mybir.dt.float32
    bf16 = mybir.dt.bfloat16

    GW = _G * K  # 512

    nc = bacc.Bacc(None)
    din_d = nc.dram_tensor("din", [N, _W], bf16, kind="ExternalInput")
    out_d = nc.dram_tensor("out", [N, N * K], bf16, kind="ExternalOutput")
    scr_d = nc.dram_tensor("scr", [N, N * K], bf16, kind="Internal")

    def op_slices(side, g):
        """(lhsT, rhs, tile_position) for side (0=term2, 1=term1), group g."""
        base = 32 * (2 * side + (g >= 8))
        col = (g % 8) * _GCOL
        return (
            din_t[base : base + _CTR, col : col + N],
            din_t[base : base + _CTR, col + N : col + _GCOL],
            (base, 0),
        )

    with tile.TileContext(nc) as tc:
        with (
            tc.tile_pool(name="persist", bufs=1) as pp,
            tc.tile_pool(name="acc", bufs=8) as ap,
            tc.tile_pool(name="rb", bufs=4) as rp,
            tc.tile_pool(name="psumA", bufs=4, space="PSUM") as qpa,
            tc.tile_pool(name="psumC", bufs=4, space="PSUM") as qpc,
        ):
            din_t = pp.tile([N, _W], bf16, tag="din")
            t1 = pp.tile([N, N * K], bf16, tag="t1")
            scr_flat = scr_d.rearrange("a b -> (a b)")

            # Streaming input: 8 column chunks (1 group per stripe each).
            for h in range(8):
                cw = _W // 8
                nc.sync.dma_start(
                    din_t[:, h * cw : (h + 1) * cw], din_d[:, h * cw : (h + 1) * cw]
                )

            # HAM warm-up: dense dummy matmuls during the input-DMA window
            # flip the PE clock gate to 2.4 GHz before phase A begins
            # (~600ns -> ~380ns per N=512 matmul).
            wi = pp.tile([32, 640], bf16, tag="warm")
            nc.gpsimd.memset(wi[:], 0.0)
            wp = qpa.tile([N, 512], fp32, tag="psA")
            for _ in range(10):
                nc.tensor.matmul(
                    wp[:], wi[:, 0:128], wi[:, 128:640],
                    start=True, stop=True, tile_position=(0, 0),
                )

            # Phase A: term1 in transposed layout [j, (i,k)] -> t1 (bf16),
            # casts alternating DVE/ACT; scatter to scratch[j, i, k] after
            # every 2 groups.
            for g in range(_NG):
                lhsT, rhs, tpos = op_slices(1, g)
                ps = qpa.tile([N, GW], fp32, tag="psA")
                nc.tensor.matmul(
                    ps[:], lhsT, rhs, start=True, stop=True, tile_position=tpos
                )
                sl = t1[:, g * GW : (g + 1) * GW]
                if g % 2 == 0:
                    nc.vector.tensor_copy(sl, ps[:])
                else:
                    nc.scalar.copy(sl, ps[:])
                if g % 2 == 1:
                    nc.sync.dma_start(
                        scr_d[:, (g - 1) * GW : (g + 1) * GW],
                        t1[:, (g - 1) * GW : (g + 1) * GW],
                    )


            # Phase C: per pair of groups: strided readback [i, (j16,k)]
            # (128B chunks); per group: term2 matmul, DVE add, relu; store.
            for gp in range(_NG // 2):
                rb = rp.tile([N, 2 * GW], bf16, tag="rb")
                rbv = rb.rearrange("a (j k) -> a j k", j=2 * _G)
                src = scr_flat.rearrange("(j i k) -> i j k", j=N, i=N, k=K)[
                    :, gp * 2 * _G : (gp + 1) * 2 * _G, :
                ]
                # Alternate the two HWDGE rings so descriptor generation for
                # consecutive readbacks proceeds in parallel.
                (nc.sync if gp % 2 == 0 else nc.scalar).dma_start(rbv[:], src)

                acc = ap.tile([N, 2 * GW], bf16, tag="acc")
                for q in range(2):
                    g = 2 * gp + q
                    lhsT, rhs, tpos = op_slices(0, g)
                    ps = qpc.tile([N, GW], fp32, tag="psC")
                    nc.tensor.matmul(
                        ps[:], lhsT, rhs, start=True, stop=True, tile_position=tpos
                    )
                    nc.vector.tensor_tensor(
                        acc[:, q * GW : (q + 1) * GW],
                        ps[:],
                        rb[:, q * GW : (q + 1) * GW],
                        mybir.AluOpType.add,
                    )
                # relu on DVE (427ns vs 1147ns on ACT at this width); output
                # DMA issued on the ACT HWDGE ring to keep SP free for rbs.
                sl = acc[:]
                nc.vector.tensor_scalar_max(sl, sl, 0.0)
                nc.scalar.dma_start(
                    out_d[:, gp * 2 * GW : (gp + 1) * 2 * GW], acc[:]
                )
    nc.finalize()
    return nc


def _host_pack(vf, d, w, b):
    """Per-batch host prepack -> list of per-core input dicts (bf16)."""
    import ml_dtypes

    bf = ml_dtypes.bfloat16
    w_i, w_j = w[:F], w[F:]
    pi = np.einsum("bncf,fk->bnck", vf, w_i, optimize=True)
    pj = np.einsum("bncf,fk->bnck", vf, w_j, optimize=True) + b

    def pack_side(dT, proj):
        """dT [(row,c), col] grouped by G rows; proj [row, c, k].

        Returns [2, 24, 5120]: two 8-group stripes, each group packed as
        [ltw_g (128 cols) | wdg_g (512 cols)].
        """
        out = np.zeros((2, _CTR, _W), np.float32)
        rr = np.arange(_G)
        for g in range(_NG):
            s, gg = divmod(g, 8)
            col = gg * _GCOL
            # ltw_g[(r*3+c), col] = d[row=g*G+r, col, c]
            out[s, :, col : col + N] = dT[g * _CTR : (g + 1) * _CTR]
            # wdg_g[(r*3+c), r*64+k] = proj[g*G+r, c, k]
            blk = np.zeros((_CTR, _G, K), np.float32)
            for c in range(C):
                blk[rr * C + c, rr, :] = proj[g * _G + rr, c, :]
            out[s, :, col + N : col + _GCOL] = blk.reshape(_CTR, _G * K)
        return out

    in_maps = []
    for bi in range(B):
        A2 = d[bi].transpose(1, 2, 0).reshape(N * C, N)  # [(j,c), i]
        A1 = d[bi].transpose(0, 2, 1).reshape(N * C, N)  # [(i,c), j]
        s2 = pack_side(A2, pj[bi])
        s1 = pack_side(A1, pi[bi])
        din = np.zeros((N, _W), np.float32)
        din[0:_CTR] = s2[0]
        din[32 : 32 + _CTR] = s2[1]
        din[64 : 64 + _CTR] = s1[0]
        din[96 : 96 + _CTR] = s1[1]
        in_maps.append({"din": np.ascontiguousarray(din).astype(bf)})
    return in_maps


def _run(in_maps, trace=False, **kw):
    from concourse.bass_utils import run_bass_kernel_spmd

    if "nc" not in _cached:
        _cached["nc"] = _build_nc()
    return run_bass_kernel_spmd(
        _cached["nc"], in_maps, core_ids=list(range(_N_CORES)), trace=trace, **kw
    )


def kernel(**inputs: np.ndarray) -> np.ndarray:
    vf = np.asarray(inputs["vector_features"], np.float32)
    d = np.asarray(inputs["distances"], np.float32)
    w = np.asarray(inputs["w_vs"], np.float32)
    b = np.asarray(inputs["b_vs"], np.float32)

    in_maps = _host_pack(vf, d, w, b)
    res = _run(in_maps)
    out = np.stack([r["out"] for r in res.results])  # [B, N, N*K] bf16
    return out.reshape(B, N, N, K).astype(np.float32)


if __name__ == "__main__":
    rng = np.random.default_rng(0)
    ins = {
        "vector_features": rng.standard_normal((B, N, C, F)).astype(np.float32),
        "distances": rng.standard_normal((B, N, N, C)).astype(np.float32),
        "w_vs": (rng.standard_normal((2 * F, K)) / np.sqrt(2 * F)).astype(np.float32),
        "b_vs": np.zeros(K, np.float32),
    }
    out = kernel(**ins)
    pi = np.einsum("bncf,fk->bnck", ins["vector_features"], ins["w_vs"][:F])
    pj = np.einsum("bncf,fk->bnck", ins["vector_features"], ins["w_vs"][F:])
    s = np.einsum("bick,bijc->bijk", pi, ins["distances"]) + np.einsum(
        "bjck,bijc->bijk", pj, ins["distances"]
    )
    want = np.maximum(s, 0)
    rel = np.abs(out - want).max() / np.abs(want).max()
    print("rel err vs numpy:", rel)
